# revision 13
# baseline (speedup 1.0000x reference)
"""AdaptiveSpectralRefinement Trainium2 kernel (8 NeuronCores, data-parallel over batch).

Key ideas:
  * rfft -> freq-weight -> mask -> irfft along the 96-long seq axis is a LINEAR
    map.  Host-side we build A_low, A_high [96,96] from freq_weights, so the
    whole spectral stage becomes two small matmuls.
  * The spectral matmul is oriented so its PSUM output IS the transposed
    activation (d on partitions) that the channel matmuls need as lhsT:
        psum[d, j] = sum_t x[t, d] * AB[t, j],  AB = [A_low^T | A_high^T | pad]
  * (batch, seq) rows are packed into 128-partition row tiles for the channel
    matmuls (PE time only depends on streamed free-size, 512 per matmul).
  * Biases enter PSUM via K=1 ones-row matmuls (outer product), not vector ops.
  * out = zh + g*(zl - zh); LayerNorm via bn_stats/bn_aggr; final scale/shift
    fused into one ScalarE activation with per-partition scale/bias.
  * float32r matmuls: 1 cycle/row at free>=256 (4x faster than plain fp32).
"""

import numpy as np

import concourse.bass as bass
import concourse.tile as tile
from concourse import bacc, mybir
from concourse.bass_utils import run_bass_kernel_spmd

# Problem constants (hardcoded per harness contract)
B = 512
S = 96
D = 512
F = S // 2 + 1  # 49
CUTOFF = F // 3  # 16
EPS = 1e-5
N_CORES = 8
B_LOC = B // N_CORES  # 64
GRP = 4  # samples per packing group
ROWS_PER_GRP = GRP * S  # 384
ROW_TILES = ROWS_PER_GRP // 128  # 3
N_GRPS = B_LOC // GRP  # 16
SPEC_FREE = 192  # spectral rhs free size (2*S, bf16 needs no pad)

F32 = mybir.dt.float32
F32R = mybir.dt.float32r
BF16 = mybir.dt.bfloat16


def _host_prep(freq_weights, Wl, bl, Wh, bh, Wg, bg):
    """Build the spectral matrices and weight layouts on the host."""
    fw = np.asarray(freq_weights, np.float64)
    eye = np.eye(S)
    RF = np.fft.rfft(eye, axis=0)  # [F, S]
    k = np.arange(F)
    wlo = fw * (k < CUTOFF)
    whi = fw * (k >= CUTOFF)
    A_low = np.fft.irfft(wlo[:, None] * RF, n=S, axis=0)  # [S, S]; xl = A_low @ x_b
    A_high = np.fft.irfft(whi[:, None] * RF, n=S, axis=0)
    import ml_dtypes

    bf16 = ml_dtypes.bfloat16
    AB = np.zeros((S, SPEC_FREE), bf16)
    AB[:, 0:S] = A_low.T.astype(bf16)
    AB[:, S : 2 * S] = A_high.T.astype(bf16)

    # The reference's gate input is [zl, zh] (post-linear), so fold:
    #   zg = zl @ Wg_top + zh @ Wg_bot + bg
    #      = xl @ (Wl Wg_top) + xh @ (Wh Wg_bot) + (bl Wg_top + bh Wg_bot + bg)
    Wl64 = np.asarray(Wl, np.float64)
    Wh64 = np.asarray(Wh, np.float64)
    Wg64 = np.asarray(Wg, np.float64)
    bl64 = np.asarray(bl, np.float64)
    bh64 = np.asarray(bh, np.float64)
    Wgl = Wl64 @ Wg64[:D]
    Wgh = Wh64 @ Wg64[D:]
    bgp = bl64 @ Wg64[:D] + bh64 @ Wg64[D:] + np.asarray(bg, np.float64)

    def chunk4(w):
        return np.ascontiguousarray(
            w.astype(bf16).reshape(4, 128, D).transpose(1, 0, 2)
        )  # [128, 4, D], chunk j = w[128j:128j+128, :]

    Wg8 = np.ascontiguousarray(
        np.concatenate([Wgl, Wgh], axis=0)
        .astype(bf16)
        .reshape(8, 128, D)
        .transpose(1, 0, 2)
    )  # chunks 0-3: Wgl (applied to xl), 4-7: Wgh (applied to xh)
    return {
        "ab": AB,
        "wl": chunk4(Wl64),
        "wh": chunk4(Wh64),
        "wg": Wg8,
        "bdp": (bh64 - bl64).astype(np.float32).reshape(1, D),
        "bl_f32": bl64.astype(np.float32).reshape(1, D),
        "bg": bgp.astype(bf16).reshape(1, D),
    }


def _build_nc(use_affine: bool):
    nc = bacc.Bacc("TRN2", target_bir_lowering=False, debug=False)

    x_d = nc.declare_dram_parameter("x", [B_LOC * S, D], BF16, isOutput=False)
    xf_d = nc.declare_dram_parameter("xf", [B_LOC * S, D], F32, isOutput=False)
    ab_d = nc.declare_dram_parameter("ab", [S, SPEC_FREE], BF16, isOutput=False)
    wl_d = nc.declare_dram_parameter("wl", [128, 4, D], BF16, isOutput=False)
    wh_d = nc.declare_dram_parameter("wh", [128, 4, D], BF16, isOutput=False)
    wg_d = nc.declare_dram_parameter("wg", [128, 8, D], BF16, isOutput=False)
    bdp_d = nc.declare_dram_parameter("bdp", [1, D], F32, isOutput=False)
    bg_d = nc.declare_dram_parameter("bg", [1, D], BF16, isOutput=False)
    ones_d = nc.declare_dram_parameter("ones", [1, 128], BF16, isOutput=False)
    if use_affine:
        gam_d = nc.declare_dram_parameter("gam", [1, D], F32, isOutput=False)
        bet_d = nc.declare_dram_parameter("bet", [1, D], F32, isOutput=False)
    out_d = nc.declare_dram_parameter("out", [B_LOC * S, D], F32, isOutput=True)

    x3 = x_d.rearrange("(b s) d -> b s d", s=S)

    from contextlib import ExitStack

    with tile.TileContext(nc) as tc, ExitStack() as ctx:
        singles = ctx.enter_context(tc.tile_pool(name="singles", bufs=1))
        xs_pool = ctx.enter_context(tc.tile_pool(name="xs", bufs=8))
        xf_pool = ctx.enter_context(tc.tile_pool(name="xf", bufs=3))
        pack_pool = ctx.enter_context(tc.tile_pool(name="pack", bufs=2))
        eplg = ctx.enter_context(tc.tile_pool(name="eplg", bufs=3))
        ypool = ctx.enter_context(tc.tile_pool(name="ypool", bufs=6))
        small = ctx.enter_context(tc.tile_pool(name="small", bufs=6))
        nwt = ctx.enter_context(tc.tile_pool(name="nwt", bufs=4))
        spec_psum = ctx.enter_context(tc.tile_pool(name="specp", bufs=2, space="PSUM"))
        mm_psum = ctx.enter_context(tc.tile_pool(name="mmp", bufs=4, space="PSUM"))

        # --- constants ---
        ab_sb = singles.tile([S, SPEC_FREE], BF16)
        nc.sync.dma_start(out=ab_sb[:], in_=ab_d[:])
        wl_sb = singles.tile([128, 4, D], BF16)
        wh_sb = singles.tile([128, 4, D], BF16)
        wg_sb = singles.tile([128, 8, D], BF16)
        bd_b = singles.tile([128, D], F32)
        bg_sb = singles.tile([1, D], BF16)
        ones_sb = singles.tile([1, 128], BF16)

        def load_weights():
            nc.sync.dma_start(out=wl_sb[:], in_=wl_d[:])
            nc.sync.dma_start(out=wh_sb[:], in_=wh_d[:])
            nc.sync.dma_start(out=wg_sb[:], in_=wg_d[:])
            nc.sync.dma_start(
                out=bd_b[:],
                in_=bass.AP(tensor=bdp_d, offset=0, ap=[[0, 128], [1, D]]),
            )
            nc.sync.dma_start(out=bg_sb[:], in_=bg_d[:])
            nc.sync.dma_start(out=ones_sb[:], in_=ones_d[:])
        magic_sb = singles.tile([128, ROW_TILES], mybir.dt.int32)
        nc.vector.memset(magic_sb[:], 0x5F3759DF)
        if use_affine:
            gam_b = singles.tile([128, D], F32)
            nc.sync.dma_start(
                out=gam_b[:],
                in_=bass.AP(tensor=gam_d, offset=0, ap=[[0, 128], [1, D]]),
            )
            bet_b = singles.tile([128, D], F32)
            nc.sync.dma_start(
                out=bet_b[:],
                in_=bass.AP(tensor=bet_d, offset=0, ap=[[0, 128], [1, D]]),
            )

        def mm(out_ap, lhsT, rhs, start, stop):
            nc.tensor.matmul(out_ap, lhsT=lhsT, rhs=rhs, start=start, stop=stop)

        packs = {}

        def emit_spectral(gi, i):
            if gi >= N_GRPS:
                return
            if i == 0:
                packs[gi] = pack_pool.tile([128, 2, 4, GRP, S], BF16, tag="pack", name=f"pack_{gi}")
            pack = packs[gi]
            b = gi * GRP + i
            xs = xs_pool.tile([S, D], BF16, tag="xs")
            nc.sync.dma_start(out=xs[:], in_=x3[b])
            spec_ps = spec_psum.tile([128, 4, SPEC_FREE], F32, tag="spec")
            for j in range(4):
                mm(
                    spec_ps[:, j, :],
                    xs[:, 128 * j : 128 * (j + 1)],
                    ab_sb[:],
                    start=True,
                    stop=True,
                )
            src_ap = spec_ps[:, :, 0 : 2 * S].rearrange("p c (l s) -> p l c s", l=2)
            dst = pack[:, :, :, i, :]
            nc.scalar.copy(out=dst, in_=src_ap)

        for i in range(GRP):
            emit_spectral(0, i)
        load_weights()

        for grp in range(N_GRPS):
            pack = packs[grp]
            packL = pack[:, 0].rearrange("p j i s -> p j (i s)")  # [128, 4, 384]
            packH = pack[:, 1].rearrange("p j i s -> p j (i s)")

            mvg = small.tile([128, 2, ROW_TILES], F32, tag="mvg")
            y_tiles = []
            for r in range(ROW_TILES):
                row0 = grp * ROWS_PER_GRP + r * 128
                xf = xf_pool.tile([128, D], F32, tag="xf")
                nc.sync.dma_start(out=xf[:], in_=xf_d[row0 : row0 + 128, :])

                zl_ps = mm_psum.tile([128, D], F32, tag="mmz", name=f"zl_{grp}_{r}")
                zh_ps = mm_psum.tile([128, D], F32, tag="mmz", name=f"zh_{grp}_{r}")
                zg_ps = mm_psum.tile([128, D], F32, tag="mmz", name=f"zg_{grp}_{r}")

                rs = slice(128 * r, 128 * (r + 1))
                # ml = sum_j lT_j @ Wl_j   (bias folded into epilogue/host)
                for j in range(4):
                    mm(zl_ps[:], packL[:, j, rs], wl_sb[:, j, :], j == 0, j == 3)
                # mh = sum_j hT_j @ Wh_j
                for j in range(4):
                    mm(zh_ps[:], packH[:, j, rs], wh_sb[:, j, :], j == 0, j == 3)
                # zg = bg + sum_j lT_j @ Wg_j + sum_j hT_j @ Wg_{4+j}
                mm(zg_ps[:], ones_sb[:], bg_sb[:], start=True, stop=False)
                for j in range(4):
                    mm(zg_ps[:], packL[:, j, rs], wg_sb[:, j, :], False, False)
                for j in range(4):
                    mm(zg_ps[:], packH[:, j, rs], wg_sb[:, 4 + j, :], False, j == 3)

                # epilogue (per row-tile part)
                g_sb = eplg.tile([128, D], F32, tag="g")
                nc.scalar.activation(
                    g_sb[:], zg_ps[:], mybir.ActivationFunctionType.Sigmoid
                )
                # zh_sb = mh + (bh - bl): bias fold rides the PSUM->SBUF move
                zh_sb = eplg.tile([128, D], F32, tag="zhs")
                nc.vector.tensor_add(zh_sb[:], zh_ps[:], bd_b[:])
                # diff = ml - zh_sb = (zl - zh) exactly
                diff = eplg.tile([128, D], F32, tag="diff")
                nc.vector.tensor_sub(diff[:], zl_ps[:], zh_sb[:])
                p_sb = eplg.tile([128, D], F32, tag="p")
                nc.vector.tensor_mul(p_sb[:], g_sb[:], diff[:])
                q_sb = eplg.tile([128, D], F32, tag="q")
                nc.gpsimd.tensor_add(q_sb[:], p_sb[:], zh_sb[:])
                # xf was pre-biased with +bl on the host
                y_sb = ypool.tile([128, D], F32, tag="y")
                y_tiles.append(y_sb)
                nc.gpsimd.tensor_add(y_sb[:], q_sb[:], xf[:])

                stats = small.tile([128, 6], F32, tag="stats")
                nc.vector.bn_stats(out=stats[:], in_=y_sb[:])
                nc.vector.bn_aggr(out=mvg[:, :, r], in_=stats[:])

                emit_spectral(grp + 1, r)

            emit_spectral(grp + 1, 3)
            del packs[grp]

            # rstd = 1/sqrt(var+eps) via magic seed + 2 Newton iterations (DVE only)
            R = ROW_TILES
            veps = nwt.tile([128, R], F32, tag="veps")
            nc.vector.tensor_scalar(
                out=veps[:], in0=mvg[:, 1, :], scalar1=EPS, scalar2=None,
                op0=mybir.AluOpType.add,
            )
            hv = nwt.tile([128, R], F32, tag="hv")
            nc.vector.tensor_scalar(
                out=hv[:], in0=veps[:], scalar1=0.5, scalar2=None,
                op0=mybir.AluOpType.mult,
            )
            ti = nwt.tile([128, R], mybir.dt.int32, tag="ti")
            nc.vector.tensor_scalar(
                out=ti[:], in0=veps[:].bitcast(mybir.dt.int32), scalar1=1,
                scalar2=None, op0=mybir.AluOpType.arith_shift_right,
            )
            yk = nwt.tile([128, R], F32, tag="yk")
            nc.vector.tensor_sub(yk[:].bitcast(mybir.dt.int32), magic_sb[:], ti[:])
            for _ in range(1):
                aa = nwt.tile([128, R], F32, tag="aa")
                nc.vector.tensor_mul(aa[:], yk[:], yk[:])
                cc = nwt.tile([128, R], F32, tag="cc")
                nc.vector.tensor_mul(cc[:], aa[:], hv[:])
                dd = nwt.tile([128, R], F32, tag="dd")
                nc.vector.tensor_scalar(
                    out=dd[:], in0=cc[:], scalar1=1.5, scalar2=-1.0,
                    op0=mybir.AluOpType.subtract, op1=mybir.AluOpType.mult,
                )
                y2 = nwt.tile([128, R], F32, tag="y2")
                nc.vector.tensor_mul(y2[:], yk[:], dd[:])
                yk = y2
            rstd3 = yk

            for r in range(ROW_TILES):
                row0 = grp * ROWS_PER_GRP + r * 128
                yn = eplg.tile([128, D], F32, tag="yn")
                nc.vector.tensor_scalar(
                    out=yn[:],
                    in0=y_tiles[r][:],
                    scalar1=mvg[:, 0, r : r + 1],
                    scalar2=rstd3[:, r : r + 1],
                    op0=mybir.AluOpType.subtract,
                    op1=mybir.AluOpType.mult,
                )
                if use_affine:
                    ya = eplg.tile([128, D], F32, tag="ya")
                    nc.vector.tensor_mul(ya[:], yn[:], gam_b[:])
                    yo = eplg.tile([128, D], F32, tag="yo")
                    nc.vector.tensor_add(yo[:], ya[:], bet_b[:])
                    nc.sync.dma_start(out=out_d[row0 : row0 + 128, :], in_=yo[:])
                else:
                    nc.sync.dma_start(out=out_d[row0 : row0 + 128, :], in_=yn[:])

    nc.finalize()
    return nc


_NC_CACHE = {}


def _in_maps(params, x):
    import ml_dtypes

    bf16 = ml_dtypes.bfloat16
    xb = x.astype(bf16)
    maps = []
    for i in range(N_CORES):
        m = {k: v for k, v in params.items() if k != "bl_f32"}
        m["x"] = np.ascontiguousarray(
            xb[i * B_LOC : (i + 1) * B_LOC].reshape(B_LOC * S, D)
        )
        m["xf"] = np.ascontiguousarray(
            (x[i * B_LOC : (i + 1) * B_LOC].reshape(B_LOC * S, D)
             + params["bl_f32"]).astype(np.float32)
        )
        m["ones"] = np.ones((1, 128), bf16)
        maps.append(m)
    return maps


def kernel(x, freq_weights, Wl, bl, Wh, bh, Wg, bg, gamma, beta):
    x = np.asarray(x, np.float32)
    gamma = np.asarray(gamma, np.float32)
    beta = np.asarray(beta, np.float32)
    use_affine = not (
        np.all(gamma == gamma.flat[0])
        and gamma.flat[0] == 1.0
        and np.all(beta == 0.0)
    )

    params = _host_prep(freq_weights, Wl, bl, Wh, bh, Wg, bg)
    if use_affine:
        params["gam"] = gamma.reshape(1, D)
        params["bet"] = beta.reshape(1, D)

    if use_affine not in _NC_CACHE:
        _NC_CACHE[use_affine] = _build_nc(use_affine)
    nc = _NC_CACHE[use_affine]

    core_ids = list(range(N_CORES))
    in_maps = _in_maps(params, x)

    res = run_bass_kernel_spmd(nc, in_maps, core_ids)
    out = np.concatenate(
        [res.results[i]["out"].reshape(B_LOC, S, D) for i in range(N_CORES)], axis=0
    )
    return out


# revision 14
# speedup vs baseline: 1.2048x; 1.2048x over previous
"""AdaptiveSpectralRefinement Trainium2 kernel (8 NeuronCores, data-parallel over batch).

Key ideas:
  * rfft -> freq-weight -> mask -> irfft along the 96-long seq axis is a LINEAR
    map.  Host-side we build A_low, A_high [96,96] from freq_weights, so the
    whole spectral stage becomes two small matmuls.
  * The spectral matmul is oriented so its PSUM output IS the transposed
    activation (d on partitions) that the channel matmuls need as lhsT:
        psum[d, j] = sum_t x[t, d] * AB[t, j],  AB = [A_low^T | A_high^T | pad]
  * (batch, seq) rows are packed into 128-partition row tiles for the channel
    matmuls (PE time only depends on streamed free-size, 512 per matmul).
  * Biases enter PSUM via K=1 ones-row matmuls (outer product), not vector ops.
  * out = zh + g*(zl - zh); LayerNorm via bn_stats/bn_aggr; final scale/shift
    fused into one ScalarE activation with per-partition scale/bias.
  * float32r matmuls: 1 cycle/row at free>=256 (4x faster than plain fp32).
"""

import numpy as np

import concourse.bass as bass
import concourse.tile as tile
from concourse import bacc, mybir
from concourse.bass_utils import run_bass_kernel_spmd

# Problem constants (hardcoded per harness contract)
B = 512
S = 96
D = 512
F = S // 2 + 1  # 49
CUTOFF = F // 3  # 16
EPS = 1e-5
N_CORES = 8
B_LOC = B // N_CORES  # 64
GRP = 4  # samples per packing group
ROWS_PER_GRP = GRP * S  # 384
ROW_TILES = ROWS_PER_GRP // 128  # 3
N_GRPS = B_LOC // GRP  # 16
SPEC_FREE = 192  # spectral rhs free size (2*S, bf16 needs no pad)

F32 = mybir.dt.float32
F32R = mybir.dt.float32r
BF16 = mybir.dt.bfloat16


def _host_prep(freq_weights, Wl, bl, Wh, bh, Wg, bg):
    """Build the spectral matrices and weight layouts on the host."""
    fw = np.asarray(freq_weights, np.float64)
    eye = np.eye(S)
    RF = np.fft.rfft(eye, axis=0)  # [F, S]
    k = np.arange(F)
    wlo = fw * (k < CUTOFF)
    whi = fw * (k >= CUTOFF)
    A_low = np.fft.irfft(wlo[:, None] * RF, n=S, axis=0)  # [S, S]; xl = A_low @ x_b
    A_high = np.fft.irfft(whi[:, None] * RF, n=S, axis=0)
    import ml_dtypes

    bf16 = ml_dtypes.bfloat16
    AB = np.zeros((S, SPEC_FREE), bf16)
    AB[:, 0:S] = A_low.T.astype(bf16)
    AB[:, S : 2 * S] = A_high.T.astype(bf16)

    # The reference's gate input is [zl, zh] (post-linear), so fold:
    #   zg = zl @ Wg_top + zh @ Wg_bot + bg
    #      = xl @ (Wl Wg_top) + xh @ (Wh Wg_bot) + (bl Wg_top + bh Wg_bot + bg)
    Wl64 = np.asarray(Wl, np.float64)
    Wh64 = np.asarray(Wh, np.float64)
    Wg64 = np.asarray(Wg, np.float64)
    bl64 = np.asarray(bl, np.float64)
    bh64 = np.asarray(bh, np.float64)
    Wgl = Wl64 @ Wg64[:D]
    Wgh = Wh64 @ Wg64[D:]
    bgp = bl64 @ Wg64[:D] + bh64 @ Wg64[D:] + np.asarray(bg, np.float64)

    def chunk4(w):
        return np.ascontiguousarray(
            w.astype(bf16).reshape(4, 128, D).transpose(1, 0, 2)
        )  # [128, 4, D], chunk j = w[128j:128j+128, :]

    Wg8 = np.ascontiguousarray(
        np.concatenate([Wgl, Wgh], axis=0)
        .astype(bf16)
        .reshape(8, 128, D)
        .transpose(1, 0, 2)
    )  # chunks 0-3: Wgl (applied to xl), 4-7: Wgh (applied to xh)
    return {
        "ab": AB,
        "wl": chunk4(Wl64),
        "wh": chunk4(Wh64),
        "wg": Wg8,
        "bdp": (bh64 - bl64).astype(np.float32).reshape(1, D),
        "bl_f32": bl64.astype(np.float32).reshape(1, D),
        "bg": bgp.astype(bf16).reshape(1, D),
    }


def _build_nc(use_affine: bool):
    nc = bacc.Bacc("TRN2", target_bir_lowering=False, debug=False)

    x_d = nc.declare_dram_parameter("x", [B_LOC * S, D], BF16, isOutput=False)
    xf_d = nc.declare_dram_parameter("xf", [B_LOC * S, D], F32, isOutput=False)
    ab_d = nc.declare_dram_parameter("ab", [S, SPEC_FREE], BF16, isOutput=False)
    wl_d = nc.declare_dram_parameter("wl", [128, 4, D], BF16, isOutput=False)
    wh_d = nc.declare_dram_parameter("wh", [128, 4, D], BF16, isOutput=False)
    wg_d = nc.declare_dram_parameter("wg", [128, 8, D], BF16, isOutput=False)
    bdp_d = nc.declare_dram_parameter("bdp", [1, D], F32, isOutput=False)
    bg_d = nc.declare_dram_parameter("bg", [1, D], BF16, isOutput=False)
    ones_d = nc.declare_dram_parameter("ones", [1, 128], BF16, isOutput=False)
    if use_affine:
        gam_d = nc.declare_dram_parameter("gam", [1, D], F32, isOutput=False)
        bet_d = nc.declare_dram_parameter("bet", [1, D], F32, isOutput=False)
    out_d = nc.declare_dram_parameter("out", [B_LOC * S, D], F32, isOutput=True)

    x3 = x_d.rearrange("(b s) d -> b s d", s=S)

    from contextlib import ExitStack

    with tile.TileContext(nc) as tc, ExitStack() as ctx:
        singles = ctx.enter_context(tc.tile_pool(name="singles", bufs=1))
        xs_pool = ctx.enter_context(tc.tile_pool(name="xs", bufs=8))
        xf_pool = ctx.enter_context(tc.tile_pool(name="xf", bufs=3))
        pack_pool = ctx.enter_context(tc.tile_pool(name="pack", bufs=2))
        eplg = ctx.enter_context(tc.tile_pool(name="eplg", bufs=3))
        ypool = ctx.enter_context(tc.tile_pool(name="ypool", bufs=6))
        small = ctx.enter_context(tc.tile_pool(name="small", bufs=6))
        nwt = ctx.enter_context(tc.tile_pool(name="nwt", bufs=4))
        spec_psum = ctx.enter_context(tc.tile_pool(name="specp", bufs=1, space="PSUM"))
        mm_psum = ctx.enter_context(tc.tile_pool(name="mmp", bufs=2, space="PSUM"))

        # --- constants ---
        ab_sb = singles.tile([S, SPEC_FREE], BF16)
        nc.sync.dma_start(out=ab_sb[:], in_=ab_d[:])
        wl_sb = singles.tile([128, 4, D], BF16)
        wh_sb = singles.tile([128, 4, D], BF16)
        wg_sb = singles.tile([128, 8, D], BF16)
        bd_b = singles.tile([128, D], F32)
        bg_sb = singles.tile([1, D], BF16)
        ones_sb = singles.tile([1, 128], BF16)

        def load_weights():
            nc.sync.dma_start(out=wl_sb[:], in_=wl_d[:])
            nc.sync.dma_start(out=wh_sb[:], in_=wh_d[:])
            nc.sync.dma_start(out=wg_sb[:], in_=wg_d[:])
            nc.sync.dma_start(
                out=bd_b[:],
                in_=bass.AP(tensor=bdp_d, offset=0, ap=[[0, 128], [1, D]]),
            )
            nc.sync.dma_start(out=bg_sb[:], in_=bg_d[:])
            nc.sync.dma_start(out=ones_sb[:], in_=ones_d[:])
        magic_sb = singles.tile([128, ROW_TILES], mybir.dt.int32)
        nc.vector.memset(magic_sb[:], 0x5F3759DF)
        if use_affine:
            gam_b = singles.tile([128, D], F32)
            nc.sync.dma_start(
                out=gam_b[:],
                in_=bass.AP(tensor=gam_d, offset=0, ap=[[0, 128], [1, D]]),
            )
            bet_b = singles.tile([128, D], F32)
            nc.sync.dma_start(
                out=bet_b[:],
                in_=bass.AP(tensor=bet_d, offset=0, ap=[[0, 128], [1, D]]),
            )

        def mm(out_ap, lhsT, rhs, start, stop):
            nc.tensor.matmul(out_ap, lhsT=lhsT, rhs=rhs, start=start, stop=stop)

        packs = {}

        def emit_spectral(gi, i):
            if gi >= N_GRPS:
                return
            if i == 0:
                packs[gi] = pack_pool.tile([128, 2, 4, GRP, S], BF16, tag="pack", name=f"pack_{gi}")
            pack = packs[gi]
            b = gi * GRP + i
            xs = xs_pool.tile([S, D], BF16, tag="xs")
            nc.sync.dma_start(out=xs[:], in_=x3[b])
            spec_ps = spec_psum.tile([128, 4, 256], F32, tag="spec")
            for j in range(4):
                mm(
                    spec_ps[:, j, 0:SPEC_FREE],
                    xs[:, 128 * j : 128 * (j + 1)],
                    ab_sb[:],
                    start=True,
                    stop=True,
                )
            src_ap = spec_ps[:, :, 0 : 2 * S].rearrange("p c (l s) -> p l c s", l=2)
            dst = pack[:, :, :, i, :]
            nc.scalar.copy(out=dst, in_=src_ap)

        for i in range(GRP):
            emit_spectral(0, i)
        load_weights()

        for grp in range(N_GRPS):
            pack = packs[grp]
            packL = pack[:, 0].rearrange("p j i s -> p j (i s)")  # [128, 4, 384]
            packH = pack[:, 1].rearrange("p j i s -> p j (i s)")

            mvg = small.tile([128, 2, ROW_TILES], F32, tag="mvg")
            y_tiles = []
            for r in range(ROW_TILES):
                row0 = grp * ROWS_PER_GRP + r * 128
                xf = xf_pool.tile([128, D], F32, tag="xf")
                nc.sync.dma_start(out=xf[:], in_=xf_d[row0 : row0 + 128, :])

                zl_ps = mm_psum.tile([128, D], F32, tag="zl", name=f"zl_{grp}_{r}")
                zh_ps = mm_psum.tile([128, D], F32, tag="zh", name=f"zh_{grp}_{r}")
                zg_ps = mm_psum.tile([128, D], F32, tag="zg", name=f"zg_{grp}_{r}")

                rs = slice(128 * r, 128 * (r + 1))
                # ml = sum_j lT_j @ Wl_j   (bias folded into epilogue/host)
                for j in range(4):
                    mm(zl_ps[:], packL[:, j, rs], wl_sb[:, j, :], j == 0, j == 3)
                # mh = sum_j hT_j @ Wh_j
                for j in range(4):
                    mm(zh_ps[:], packH[:, j, rs], wh_sb[:, j, :], j == 0, j == 3)
                # zg = bg + sum_j lT_j @ Wg_j + sum_j hT_j @ Wg_{4+j}
                mm(zg_ps[:], ones_sb[:], bg_sb[:], start=True, stop=False)
                for j in range(4):
                    mm(zg_ps[:], packL[:, j, rs], wg_sb[:, j, :], False, False)
                for j in range(4):
                    mm(zg_ps[:], packH[:, j, rs], wg_sb[:, 4 + j, :], False, j == 3)

                # epilogue (per row-tile part)
                g_sb = eplg.tile([128, D], F32, tag="g")
                nc.scalar.activation(
                    g_sb[:], zg_ps[:], mybir.ActivationFunctionType.Sigmoid
                )
                # zh_sb = mh + (bh - bl): bias fold rides the PSUM->SBUF move
                zh_sb = eplg.tile([128, D], F32, tag="zhs")
                nc.vector.tensor_add(zh_sb[:], zh_ps[:], bd_b[:])
                # diff = ml - zh_sb = (zl - zh) exactly
                diff = eplg.tile([128, D], F32, tag="diff")
                nc.vector.tensor_sub(diff[:], zl_ps[:], zh_sb[:])
                p_sb = eplg.tile([128, D], F32, tag="p")
                nc.vector.tensor_mul(p_sb[:], g_sb[:], diff[:])
                q_sb = eplg.tile([128, D], F32, tag="q")
                nc.gpsimd.tensor_add(q_sb[:], p_sb[:], zh_sb[:])
                # xf was pre-biased with +bl on the host
                y_sb = ypool.tile([128, D], F32, tag="y")
                y_tiles.append(y_sb)
                nc.gpsimd.tensor_add(y_sb[:], q_sb[:], xf[:])

                stats = small.tile([128, 6], F32, tag="stats")
                nc.vector.bn_stats(out=stats[:], in_=y_sb[:])
                nc.vector.bn_aggr(out=mvg[:, :, r], in_=stats[:])

                emit_spectral(grp + 1, r)

            emit_spectral(grp + 1, 3)
            del packs[grp]

            # rstd = 1/sqrt(var+eps) via magic seed + 2 Newton iterations (DVE only)
            R = ROW_TILES
            veps = nwt.tile([128, R], F32, tag="veps")
            nc.vector.tensor_scalar(
                out=veps[:], in0=mvg[:, 1, :], scalar1=EPS, scalar2=None,
                op0=mybir.AluOpType.add,
            )
            hv = nwt.tile([128, R], F32, tag="hv")
            nc.vector.tensor_scalar(
                out=hv[:], in0=veps[:], scalar1=0.5, scalar2=None,
                op0=mybir.AluOpType.mult,
            )
            ti = nwt.tile([128, R], mybir.dt.int32, tag="ti")
            nc.vector.tensor_scalar(
                out=ti[:], in0=veps[:].bitcast(mybir.dt.int32), scalar1=1,
                scalar2=None, op0=mybir.AluOpType.arith_shift_right,
            )
            yk = nwt.tile([128, R], F32, tag="yk")
            nc.vector.tensor_sub(yk[:].bitcast(mybir.dt.int32), magic_sb[:], ti[:])
            for _ in range(1):
                aa = nwt.tile([128, R], F32, tag="aa")
                nc.vector.tensor_mul(aa[:], yk[:], yk[:])
                cc = nwt.tile([128, R], F32, tag="cc")
                nc.vector.tensor_mul(cc[:], aa[:], hv[:])
                dd = nwt.tile([128, R], F32, tag="dd")
                nc.vector.tensor_scalar(
                    out=dd[:], in0=cc[:], scalar1=1.5, scalar2=-1.0,
                    op0=mybir.AluOpType.subtract, op1=mybir.AluOpType.mult,
                )
                y2 = nwt.tile([128, R], F32, tag="y2")
                nc.vector.tensor_mul(y2[:], yk[:], dd[:])
                yk = y2
            rstd3 = yk

            for r in range(ROW_TILES):
                row0 = grp * ROWS_PER_GRP + r * 128
                yn = eplg.tile([128, D], F32, tag="yn")
                nc.vector.tensor_scalar(
                    out=yn[:],
                    in0=y_tiles[r][:],
                    scalar1=mvg[:, 0, r : r + 1],
                    scalar2=rstd3[:, r : r + 1],
                    op0=mybir.AluOpType.subtract,
                    op1=mybir.AluOpType.mult,
                )
                if use_affine:
                    ya = eplg.tile([128, D], F32, tag="ya")
                    nc.vector.tensor_mul(ya[:], yn[:], gam_b[:])
                    yo = eplg.tile([128, D], F32, tag="yo")
                    nc.vector.tensor_add(yo[:], ya[:], bet_b[:])
                    nc.sync.dma_start(out=out_d[row0 : row0 + 128, :], in_=yo[:])
                else:
                    nc.sync.dma_start(out=out_d[row0 : row0 + 128, :], in_=yn[:])

    nc.finalize()
    return nc


_NC_CACHE = {}


def _in_maps(params, x):
    import ml_dtypes

    bf16 = ml_dtypes.bfloat16
    xb = x.astype(bf16)
    maps = []
    for i in range(N_CORES):
        m = {k: v for k, v in params.items() if k != "bl_f32"}
        m["x"] = np.ascontiguousarray(
            xb[i * B_LOC : (i + 1) * B_LOC].reshape(B_LOC * S, D)
        )
        m["xf"] = np.ascontiguousarray(
            (x[i * B_LOC : (i + 1) * B_LOC].reshape(B_LOC * S, D)
             + params["bl_f32"]).astype(np.float32)
        )
        m["ones"] = np.ones((1, 128), bf16)
        maps.append(m)
    return maps


def kernel(x, freq_weights, Wl, bl, Wh, bh, Wg, bg, gamma, beta):
    x = np.asarray(x, np.float32)
    gamma = np.asarray(gamma, np.float32)
    beta = np.asarray(beta, np.float32)
    use_affine = not (
        np.all(gamma == gamma.flat[0])
        and gamma.flat[0] == 1.0
        and np.all(beta == 0.0)
    )

    params = _host_prep(freq_weights, Wl, bl, Wh, bh, Wg, bg)
    if use_affine:
        params["gam"] = gamma.reshape(1, D)
        params["bet"] = beta.reshape(1, D)

    if use_affine not in _NC_CACHE:
        _NC_CACHE[use_affine] = _build_nc(use_affine)
    nc = _NC_CACHE[use_affine]

    core_ids = list(range(N_CORES))
    in_maps = _in_maps(params, x)

    res = run_bass_kernel_spmd(nc, in_maps, core_ids)
    out = np.concatenate(
        [res.results[i]["out"].reshape(B_LOC, S, D) for i in range(N_CORES)], axis=0
    )
    return out
